# revision 24
# baseline (speedup 1.0000x reference)
"""Multi-head attention (B=2, S=2048, D=1024, H=16, d_k=64) on 8 NeuronCores.

Sharding: 8 cores = 2 batches x 4 head-groups (4 heads each).
Core c handles batch b = c//4 and heads 4*(c%4) .. 4*(c%4)+4 (feature
slice of width F=256). Each core computes its partial output-projection
contribution [S, D]; the host sums the 4 head-group partials per batch
and adds b4 (the "all-reduce" of the row-sharded W4 projection).

Device dataflow works in a "transposed world" so every matmul operand
is in its natural PE layout (contraction on partitions), with zero
on-device transposes:
  qT = W1g @ x_q.T  [F, S]
  kT = W2g @ x_k.T  [F, S]
  v  = x_v @ W3i    [S, 260]  (260 = 4 heads x (64 v cols + 1 ones col);
                               the ones col comes from the bias matmul
                               with b3i[h*65+64] = 1, W3i zero there)
  scoresT_h = kT_h.T @ qT_h   [S_keys, S_q]  (K = d_k = 64)
  attnT = exp(scoresT / 8)    ACT, PSUM->SBUF bf16, no max subtraction
  pv = v_ext.T @ attnT        [65, S_q]; row 64 = softmax denominator
  outT_h = pv[0:64] * (1/pv[64])   (reciprocal_approx_fast + gpsimd
                                    partition_broadcast + one DVE mul)
  partial = outT.T @ W4g.T    [S, D]

All matmuls bf16 with f32 PSUM accumulation.

Schedule (built to keep the PE stream dense so the HAM clock gate stays
at 8/8, and to hide everything under the ACT exp roofline):
  - prioritized chunked DMA: w1 + xq first (512-col chunks) so the
    first projection matmuls issue ~5us in
  - upfront PE work: only the m=0 half of the q/k projections (enough
    for head-pair 0's windows)
  - window (0,0): scores/exp/PV with the v-projection interleaved as
    PE filler (one s-tile per kt step, sharing the "sc" psum tag)
  - window (0,1): m=1 q/k projection groups as filler
  - window (1,0): no filler
  - window (1,1): W4 output projection for qw=0 as filler
  - tail: W4 for qw=1
  PV lags scores by 2 key tiles so PV never waits on the exp of the
  same step (ACT latency hidden at any clock).  PSUM: scores 2x2 banks
  (tag sc, shared by all filler psum) + PV accumulators 2x2 banks.
"""

import numpy as np
import ml_dtypes

import concourse.bass as bass
import concourse.mybir as mybir
import concourse.tile as tile
from concourse import bacc
from concourse.bass_utils import run_bass_kernel_spmd

BF16 = ml_dtypes.bfloat16
F32 = mybir.dt.float32
BF = mybir.dt.bfloat16

B, S, D = 2, 2048, 1024
H_CORE = 4          # heads per core
DK = 64             # head dim
F = H_CORE * DK     # features per core = 256
P = 128             # partitions
KB = D // P         # k blocks in D contraction = 8
SM = S // P         # seq tiles of 128 = 16
QW = 1024           # query window width
NQW = S // QW       # query windows = 2
VW = H_CORE * (DK + 1)  # 260: v with interleaved ones columns
N_CORES = 8


def _build_kernel():
    nc = bacc.Bacc(
        "TRN2",
        target_bir_lowering=False,
        debug=False,
        enable_asserts=False,
        num_devices=N_CORES,
    )

    xq = nc.dram_tensor("xq_t", [D, S], BF, kind="ExternalInput").ap()
    xk = nc.dram_tensor("xk_t", [D, S], BF, kind="ExternalInput").ap()
    xv = nc.dram_tensor("xv_t", [D, S], BF, kind="ExternalInput").ap()
    w1 = nc.dram_tensor("w1t", [D, F], BF, kind="ExternalInput").ap()
    w2 = nc.dram_tensor("w2t", [D, F], BF, kind="ExternalInput").ap()
    w3 = nc.dram_tensor("w3i", [D, VW], BF, kind="ExternalInput").ap()
    w4 = nc.dram_tensor("w4t", [F, D], BF, kind="ExternalInput").ap()
    b1 = nc.dram_tensor("b1c", [P, F // P], F32, kind="ExternalInput").ap()
    b2 = nc.dram_tensor("b2c", [P, F // P], F32, kind="ExternalInput").ap()
    b3 = nc.dram_tensor("b3i", [1, VW], BF, kind="ExternalInput").ap()
    out = nc.dram_tensor("out", [S, D], BF, kind="ExternalOutput").ap()

    with tile.TileContext(nc) as tc:
        _body(tc, xq, xk, xv, w1, w2, w3, w4, b1, b2, b3, out)

    nc.compile()
    return nc


def _body(tc, xq, xk, xv, w1, w2, w3, w4, b1, b2, b3, out):
    nc = tc.nc
    MF = F // P  # m tiles for the F=256 feature dim = 2

    with (
        tc.tile_pool(name="wpool", bufs=1) as wpool,
        tc.tile_pool(name="xt", bufs=24) as xt_pool,
        tc.tile_pool(name="persist", bufs=1) as persist,
        tc.tile_pool(name="attn", bufs=10) as attn_pool,
        tc.tile_pool(name="small", bufs=3) as small,
        tc.tile_pool(name="stage", bufs=2) as stage,
        tc.tile_pool(name="psum", bufs=1, space="PSUM") as psum,
    ):
        # ---- SBUF tiles; DMA issued in priority order ----
        w1_sb = [wpool.tile([P, F], BF, name=f"w1_{k}", tag=f"w1_{k}") for k in range(KB)]
        w2_sb = [wpool.tile([P, F], BF, name=f"w2_{k}", tag=f"w2_{k}") for k in range(KB)]
        w3_sb = [wpool.tile([P, VW], BF, name=f"w3_{k}", tag=f"w3_{k}") for k in range(KB)]
        w4_sb = [wpool.tile([P, D], BF, name=f"w4_{k}", tag=f"w4_{k}") for k in range(MF)]
        xq_sb = [xt_pool.tile([P, S], BF, name=f"xq_{k}", tag="xt") for k in range(KB)]
        xk_sb = [xt_pool.tile([P, S], BF, name=f"xk_{k}", tag="xt") for k in range(KB)]
        xv_sb = [xt_pool.tile([P, S], BF, name=f"xv_{k}", tag="xt") for k in range(KB)]
        b1_sb = wpool.tile([P, MF], F32, name="b1_sb", tag="b1_sb")
        b2_sb = wpool.tile([P, MF], F32, name="b2_sb", tag="b2_sb")
        b3_sb = wpool.tile([1, VW], BF, name="b3_sb", tag="b3_sb")
        ones_row = wpool.tile([1, P], BF, name="ones_row", tag="ones_row")
        nc.vector.memset(ones_row[:], 1.0)

        # two parallel HWDGE queues: sync carries the big x tensors in
        # criticality order, scalar carries all weights + biases
        for k in range(KB):
            nc.scalar.dma_start(w1_sb[k][:], w1[k * P:(k + 1) * P, :])
        for k in range(KB):
            nc.sync.dma_start(xq_sb[k][:], xq[k * P:(k + 1) * P, :])
        for k in range(KB):
            nc.scalar.dma_start(w2_sb[k][:], w2[k * P:(k + 1) * P, :])
        for k in range(KB):
            nc.scalar.dma_start(w3_sb[k][:], w3[k * P:(k + 1) * P, :])
        for k in range(KB):
            nc.sync.dma_start(xk_sb[k][:], xk[k * P:(k + 1) * P, :])
        for k in range(KB):
            nc.sync.dma_start(xv_sb[k][:], xv[k * P:(k + 1) * P, :])
        for k in range(MF):
            nc.scalar.dma_start(w4_sb[k][:], w4[k * P:(k + 1) * P, :])
        nc.scalar.dma_start(b1_sb[:], b1[:])
        nc.scalar.dma_start(b2_sb[:], b2[:])
        nc.scalar.dma_start(b3_sb[:], b3[:])

        # preload the ACT exp table off the critical path (first real exp
        # would otherwise pay the ~2.7us table-load inside window 1)
        warm_act = wpool.tile([1, 16], BF, name="warm_act", tag="warm_act")
        nc.scalar.activation(warm_act[:], ones_row[:, 0:16],
                             mybir.ActivationFunctionType.Exp, scale=1.0)

        # persistent activations
        qT = [persist.tile([P, S], BF, name=f"qT_{m}", tag=f"qT_{m}") for m in range(MF)]
        kT = [persist.tile([P, S], BF, name=f"kT_{m}", tag=f"kT_{m}") for m in range(MF)]
        v_sb = [persist.tile([P, VW], BF, name=f"v_{s}", tag=f"v_{s}") for s in range(SM)]
        outT = [persist.tile([P, S], BF, name=f"outT_{m}", tag=f"outT_{m}") for m in range(MF)]

        # ---- one q/k projection group: dst[m][:, 512-col slice] ----
        def proj_qk(name, x_sb, w_sb, b_sb, dst, m, h):
            csl = slice(h * 512, (h + 1) * 512)
            ps = psum.tile([P, 512], F32, name=f"pp_{name}_{m}_{h}", tag="sc", bufs=2)
            for k in range(KB):
                nc.tensor.matmul(
                    ps[:],
                    w_sb[k][:, m * P:(m + 1) * P],
                    x_sb[k][:, csl],
                    start=(k == 0),
                    stop=(k == KB - 1),
                )
            nc.vector.tensor_scalar_add(dst[m][:, csl], ps[:], b_sb[:, m:m + 1])

        # upfront: m=0 projections of q and k, k-major over 4 concurrent
        # [P,512] psum groups so each x tile is consumed as its DMA lands
        def proj_m0_kmajor(name, x_sb, w_sb, b_sb, dst):
            tags = ["sc", "sc", "pv", "pv"]
            pss = [
                psum.tile([P, 512], F32, name=f"pp0_{name}_{h}", tag=tags[h], bufs=2)
                for h in range(4)
            ]
            for k in range(KB):
                for h in range(4):
                    nc.tensor.matmul(
                        pss[h][:],
                        w_sb[k][:, 0:P],
                        x_sb[k][:, h * 512:(h + 1) * 512],
                        start=(k == 0),
                        stop=(k == KB - 1),
                    )
            for h in range(4):
                nc.vector.tensor_scalar_add(
                    dst[0][:, h * 512:(h + 1) * 512], pss[h][:], b_sb[:, 0:1]
                )

        proj_m0_kmajor("q", xq_sb, w1_sb, b1_sb, qT)
        # m=1 q projection here: dense PE work that only needs xq, covering
        # the wait for the xk DMA before the k projections can run
        for h in range(4):
            proj_qk("q1", xq_sb, w1_sb, b1_sb, qT, 1, h)
        proj_m0_kmajor("k", xk_sb, w2_sb, b2_sb, kT)

        # ---- filler generators (consumed inside windows at PE idle slots) ----
        def gen_vproj():
            for s in range(SM):
                ps = psum.tile([P, VW], F32, name=f"pv_{s}", tag="sc", bufs=2)
                for k in range(KB):
                    nc.tensor.matmul(
                        ps[:],
                        xv_sb[k][:, s * P:(s + 1) * P],
                        w3_sb[k][:],
                        start=(k == 0),
                        stop=False,
                    )
                nc.tensor.matmul(ps[:], ones_row[:], b3_sb[:], start=False, stop=True)
                nc.vector.tensor_copy(v_sb[s][:], ps[:])
                yield

        def gen_m1proj():
            for h in range(4):
                proj_qk("k1", xk_sb, w2_sb, b2_sb, kT, 1, h)
                yield

        def gen_w4(qts, alt_copy=False):
            for i, qt in enumerate(qts):
                ps = psum.tile([P, D], F32, name=f"po_{qt}", tag="sc", bufs=2)
                for oc in range(D // 512):
                    for m in range(MF):
                        nc.tensor.matmul(
                            ps[:, oc * 512:(oc + 1) * 512],
                            outT[m][:, qt * P:(qt + 1) * P],
                            w4_sb[m][:, oc * 512:(oc + 1) * 512],
                            start=(m == 0),
                            stop=(m == MF - 1),
                        )
                ob = stage.tile([P, D], BF, name=f"ob_{qt}", tag="ob")
                if alt_copy and i % 2 == 1:
                    nc.scalar.copy(ob[:], ps[:])
                else:
                    nc.vector.tensor_copy(ob[:], ps[:])
                nc.sync.dma_start(out[qt * P:(qt + 1) * P, :], ob[:])
                yield

        # ---- attention window: head-pair hp, query window qw.
        #      scores(kt) / PV(kt-2) interleave; filler consumed each step. ----
        def window(hp, qw, filler=None, fill_every=1, fill_start=0, drain=False,
                   lag=2):
            qsl = slice(qw * QW, (qw + 1) * QW)
            attn_t = [[None] * SM for _ in range(2)]
            pv_ps = [
                psum.tile([P, QW], F32, name=f"pvps_{hp}_{qw}_{h2}", tag="pv", bufs=2)
                for h2 in range(2)
            ]

            def emit_scores(kt):
                for h2 in range(2):
                    rsl = slice(h2 * DK, (h2 + 1) * DK)
                    ps = psum.tile([P, QW], F32, name=f"sc_{hp}_{qw}_{kt}_{h2}",
                                   tag="sc", bufs=2)
                    for half in range(2):
                        nc.tensor.matmul(
                            ps[:, half * 512:(half + 1) * 512],
                            kT[hp][rsl, kt * P:(kt + 1) * P],
                            qT[hp][rsl, qw * QW + half * 512: qw * QW + (half + 1) * 512],
                            start=True,
                            stop=True,
                        )
                    at = attn_pool.tile([P, QW], BF, name=f"at_{hp}_{qw}_{kt}_{h2}",
                                        tag="attnT", bufs=10)
                    nc.scalar.activation(
                        at[:], ps[:], mybir.ActivationFunctionType.Exp,
                        scale=1.0 / np.sqrt(DK),
                    )
                    attn_t[h2][kt] = at

            def emit_pv(kt, h2s=(0, 1)):
                for h2 in h2s:
                    h = hp * 2 + h2
                    vsl = slice(h * (DK + 1), h * (DK + 1) + DK + 1)
                    for half in range(2):
                        nc.tensor.matmul(
                            pv_ps[h2][0:DK + 1, half * 512:(half + 1) * 512],
                            v_sb[kt][:, vsl],
                            attn_t[h2][kt][:, half * 512:(half + 1) * 512],
                            start=(kt == 0),
                            stop=(kt == SM - 1),
                        )

            def norm_all():
                # half-major phases: the half=0 chains of BOTH heads complete
                # first (they unblock the first W4 tail tiles), DVE and
                # gpsimd pipelining across phases
                recs, bcs = {}, {}

                def dens_recs(half):
                    hsl = slice(half * 512, (half + 1) * 512)
                    for h2 in range(2):
                        den = small.tile([1, 512], F32,
                                         name=f"den_{hp}_{qw}_{h2}_{half}",
                                         tag="den", bufs=4)
                        nc.vector.tensor_copy(den[:], pv_ps[h2][DK:DK + 1, hsl])
                        rec = small.tile([1, 512], F32,
                                         name=f"rec_{hp}_{qw}_{h2}_{half}",
                                         tag="rec", bufs=4)
                        nc.vector.reciprocal_approx_fast(rec[:], den[:])
                        recs[h2, half] = rec

                def bcasts(half):
                    for h2 in range(2):
                        bc = small.tile([DK, 512], F32,
                                        name=f"bc_{hp}_{qw}_{h2}_{half}",
                                        tag="bc", bufs=4)
                        nc.gpsimd.partition_broadcast(bc[:], recs[h2, half][:])
                        bcs[h2, half] = bc

                def muls(half):
                    hsl = slice(half * 512, (half + 1) * 512)
                    osl = slice(qw * QW + half * 512, qw * QW + (half + 1) * 512)
                    for h2 in range(2):
                        nc.vector.tensor_mul(
                            outT[hp][h2 * DK:(h2 + 1) * DK, osl],
                            pv_ps[h2][0:DK, hsl], bcs[h2, half][:]
                        )

                dens_recs(0)
                bcasts(0)
                dens_recs(1)
                muls(0)
                bcasts(1)
                muls(1)

            for kt in range(SM):
                emit_scores(kt)
                if (filler is not None and kt >= fill_start
                        and (kt - fill_start) % fill_every == 0):
                    next(filler, None)
                if kt >= lag:
                    emit_pv(kt - lag)
            if drain and filler is not None:
                for _ in filler:
                    pass
            for kt in range(SM - lag, SM):
                emit_pv(kt)
            norm_all()

        window(0, 0, filler=gen_vproj(), fill_every=1, fill_start=2, drain=True, lag=4)
        window(0, 1, filler=gen_m1proj(), fill_every=2, fill_start=1)
        window(1, 0)
        window(1, 1, filler=gen_w4(range(SM // 2)), fill_every=2, fill_start=2,
               drain=True)
        for _ in gen_w4(range(SM // 2, SM), alt_copy=True):
            pass


_NC_CACHE = None


def _get_nc():
    global _NC_CACHE
    if _NC_CACHE is None:
        _NC_CACHE = _build_kernel()
    return _NC_CACHE


def _make_in_maps(query, key, value, W1, b1, W2, b2, W3, b3, W4, b4):
    in_maps = []
    for c in range(N_CORES):
        b, g = divmod(c, 4)
        gs = slice(g * F, (g + 1) * F)
        w3g = W3[gs, :].T.astype(np.float32)          # [D, F]
        w3i = np.zeros((D, VW), np.float32)
        b3g = b3[gs].astype(np.float32)
        b3i = np.zeros((VW,), np.float32)
        for h in range(H_CORE):
            w3i[:, h * (DK + 1): h * (DK + 1) + DK] = w3g[:, h * DK:(h + 1) * DK]
            b3i[h * (DK + 1): h * (DK + 1) + DK] = b3g[h * DK:(h + 1) * DK]
            b3i[h * (DK + 1) + DK] = 1.0
        in_maps.append({
            "xq_t": np.ascontiguousarray(query[b].T).astype(BF16),
            "xk_t": np.ascontiguousarray(key[b].T).astype(BF16),
            "xv_t": np.ascontiguousarray(value[b].T).astype(BF16),
            "w1t": np.ascontiguousarray(W1[gs, :].T).astype(BF16),
            "w2t": np.ascontiguousarray(W2[gs, :].T).astype(BF16),
            "w3i": np.ascontiguousarray(w3i).astype(BF16),
            "w4t": np.ascontiguousarray(W4[:, gs].T).astype(BF16),
            "b1c": np.ascontiguousarray(b1[gs].reshape(F // P, P).T).astype(np.float32),
            "b2c": np.ascontiguousarray(b2[gs].reshape(F // P, P).T).astype(np.float32),
            "b3i": b3i.reshape(1, VW).astype(BF16),
        })
    return in_maps


def kernel(query, key, value, W1, b1, W2, b2, W3, b3, W4, b4, _trace=False, _tmpdir=None):
    args = [np.asarray(a) for a in (query, key, value, W1, b1, W2, b2, W3, b3, W4, b4)]
    nc = _get_nc()
    in_maps = _make_in_maps(*args)
    res = run_bass_kernel_spmd(
        nc, in_maps, core_ids=list(range(N_CORES)),
        trace=_trace, tmpdir=_tmpdir,
    )
    b4_f = args[10].astype(np.float32)
    full = np.zeros((B, S, D), np.float32)
    for c in range(N_CORES):
        full[c // 4] += res.results[c]["out"]
    full += b4_f[None, None, :]
    kernel.last_results = res
    return full
